# revision 7
# baseline (speedup 1.0000x reference)
"""E(3)-equivariant graph convolution (e3nn-style) on 8 Trainium2 NeuronCores.

Contract: kernel(**inputs) takes the FULL inputs of nn_E3Conv (f_in [N,16],
pos [N,3], edge_src [E], edge_dst [E], W1 [8,64], W2 [64,17]) and returns the
FULL output [N,16] float32.

Sharding (from the sharding hint, hardcoded): nodes are sharded contiguously
across the 8 cores (2560 per core); each edge is assigned to the core owning
its destination node; the host materializes per-edge source records (the
halo-exchange/gather step of a distributed GNN) sorted by destination block;
each core computes per-edge geometry, spherical harmonics, radial weights and
the tensor product on device and scatter-adds into its local nodes.

Device pipeline per 128-node block (36 edge-tiles of 128 edges):
  - geometry + raw spherical harmonics on VectorE,
  - the radial weight function (a function of one scalar r) evaluated as a
    32-center gaussian RBF fit of the reference MLP: PE transpose of r,
    ScalarE Square+Exp for the gaussians, PE matmul against the fitted
    coefficients, PE transpose back,
  - tensor-product expansion columns W[e, 243] = f_i * sh_j * w_p as four
    grouped elementwise products on VectorE (bf16),
  - scatter-add via a selection-matrix matmul (SEL[128e,128n]^T @ W) that
    accumulates all 36 tiles of a block into one PSUM tile G[128n, 243],
  - constant fold G @ F -> out[128, 16] via PE transposes + matmul.
"""

import numpy as np
from contextlib import ExitStack
from math import factorial

# ----------------------------------------------------------------------------
# e3nn constants (replicated from the reference definition; self-contained)
# ----------------------------------------------------------------------------
MAX_RADIUS = 6.0
N_BASIS = 8
N_NODES = 20000
SILU_MOM = 1.6790

SL = [(0, 1), (1, 4), (4, 9), (9, 16)]
PATHS = [(0, 0, 0), (0, 1, 1), (0, 2, 2),
         (1, 0, 1), (1, 1, 0), (1, 1, 2), (1, 2, 1), (1, 2, 3),
         (2, 0, 2), (2, 1, 1), (2, 1, 3), (2, 2, 0), (2, 2, 2),
         (3, 0, 3), (3, 1, 2), (3, 2, 1), (3, 2, 3)]
_K_OUT = {0: 3, 1: 5, 2: 5, 3: 4}
PATH_W = [float(np.sqrt((2 * l3 + 1) / _K_OUT[l3])) for (_, _, l3) in PATHS]


def _cg(j1, m1, j2, m2, j3, m3):
    if m1 + m2 != m3:
        return 0.0
    f = factorial
    pre = ((2 * j3 + 1) * f(j3 + j1 - j2) * f(j3 - j1 + j2) * f(j1 + j2 - j3) / f(j1 + j2 + j3 + 1)) ** 0.5
    pre *= (f(j3 + m3) * f(j3 - m3) * f(j1 - m1) * f(j1 + m1) * f(j2 - m2) * f(j2 + m2)) ** 0.5
    s = 0.0
    for k in range(max(0, j2 - j3 - m1, j1 - j3 + m2), min(j1 + j2 - j3, j1 - m1, j2 + m2) + 1):
        s += (-1.0) ** k / (f(k) * f(j1 + j2 - j3 - k) * f(j1 - m1 - k) * f(j2 + m2 - k) * f(j3 - j2 + m1 + k) * f(j3 - j1 - m2 + k))
    return pre * s


def _q(l):
    q = np.zeros((2 * l + 1, 2 * l + 1), dtype=np.complex128)
    for m in range(-l, 0):
        q[l + m, l + abs(m)] = 2 ** -0.5
        q[l + m, l - abs(m)] = -1j * 2 ** -0.5
    q[l, l] = 1.0
    for m in range(1, l + 1):
        q[l + m, l + abs(m)] = (-1) ** m * 2 ** -0.5
        q[l + m, l - abs(m)] = 1j * (-1) ** m * 2 ** -0.5
    return (-1j) ** l * q


def _w3j(l1, l2, l3):
    C = np.zeros((2 * l1 + 1, 2 * l2 + 1, 2 * l3 + 1))
    for m1 in range(-l1, l1 + 1):
        for m2 in range(-l2, l2 + 1):
            m3 = m1 + m2
            if -l3 <= m3 <= l3:
                C[m1 + l1, m2 + l2, m3 + l3] = _cg(l1, m1, l2, m2, l3, m3)
    Q1, Q2, Q3 = _q(l1), _q(l2), _q(l3)
    Ra = np.einsum('ai,bj,ck,ijk->abc', Q1, Q2, np.conj(Q3), C.astype(np.complex128))
    Rb = np.einsum('ai,bj,ck,ijk->abc', Q1, Q2, Q3, C.astype(np.complex128))
    R = Ra if np.abs(Ra.imag).sum() <= np.abs(Rb.imag).sum() else Rb
    R = R.real
    return (R / np.linalg.norm(R)).astype(np.float64)


W3J = {p: _w3j(*p) for p in set(PATHS)}

# device sh (raw) = [1, uy, uz, ux, ux*uy, uy*uz, 3uz^2-1, uz*ux, ux^2-uy^2]
_s3, _s5, _s15 = np.sqrt(3.0), np.sqrt(5.0), np.sqrt(15.0)
SH_CONST = np.array([1.0, _s3, _s3, _s3, _s15, _s15, _s5 / 2.0, _s15, _s15 / 2.0])

# ----------------------------------------------------------------------------
# Sharding / layout geometry
# ----------------------------------------------------------------------------
NCORES = 8
NB = 128
BLOCKS = 20
NODES_PER_CORE = NB * BLOCKS          # 2560
NPAD = NCORES * NODES_PER_CORE        # 20480
T_B = 36                              # edge tiles per node block (padded)
RG = 4                                # tiles per radial transpose group
NGROUPS = T_B // RG                   # 9
M_RBF = 32
PAD_DREL = 200.0

SHW_COLS = []
SHW_OFF = []
for _pi, (_l1, _l2, _l3) in enumerate(PATHS):
    SHW_OFF.append(len(SHW_COLS))
    for _j in range(2 * _l2 + 1):
        SHW_COLS.append((_pi, _j))
N_SHW = len(SHW_COLS)                 # 57

L1_PATHS = {l1: [pi for pi, p in enumerate(PATHS) if p[0] == l1] for l1 in range(4)}
S_L1 = {l1: sum(2 * PATHS[pi][1] + 1 for pi in L1_PATHS[l1]) for l1 in range(4)}
SHW_L1_START = {l1: SHW_OFF[L1_PATHS[l1][0]] for l1 in range(4)}
W_COLS = sum((2 * l1 + 1) * S_L1[l1] for l1 in range(4))  # 243


def _build_fold(num_neighbors):
    F = np.zeros((W_COLS, 16), dtype=np.float64)
    col = 0
    for l1 in range(4):
        for i in range(2 * l1 + 1):
            for pi in L1_PATHS[l1]:
                p_l1, p_l2, p_l3 = PATHS[pi]
                C = W3J[(p_l1, p_l2, p_l3)]
                for j in range(2 * p_l2 + 1):
                    jg = SL[p_l2][0] + j
                    for k in range(2 * p_l3 + 1):
                        kg = SL[p_l3][0] + k
                        F[col, kg] += C[i, j, k] * SH_CONST[jg]
                    col += 1
    assert col == W_COLS
    return (F / np.sqrt(num_neighbors)).astype(np.float32)


# ----------------------------------------------------------------------------
# Radial RBF fit
# ----------------------------------------------------------------------------
RBF_LO, RBF_HI = -0.3, 6.9
RBF_CEN = np.linspace(RBF_LO, RBF_HI, M_RBF)
RBF_H = (RBF_CEN[1] - RBF_CEN[0]) * 1.45


def _radial_exact(r, W1, W2):
    centers = np.linspace(0.0, MAX_RADIUS, N_BASIS + 2, dtype=np.float32)[1:-1]
    step = np.float32(MAX_RADIUS / (N_BASIS + 1))
    emb = np.exp(-((r[:, None] - centers) / step) ** 2) / 1.12 * np.float32(np.sqrt(N_BASIS))
    z = emb @ W1 / np.float32(np.sqrt(N_BASIS))
    h = SILU_MOM * (z / (1.0 + np.exp(-z)))
    return h @ W2 / np.float32(np.sqrt(W1.shape[1]))


def _fit_rbf(W1, W2):
    g = np.linspace(0.0, 6.6, 6000)
    tgt = _radial_exact(g.astype(np.float32), W1, W2).astype(np.float64)
    tgt = tgt * np.asarray(PATH_W)[None, :]
    B = np.exp(-((g[:, None] - RBF_CEN) / RBF_H) ** 2)
    C, *_ = np.linalg.lstsq(B, tgt, rcond=None)
    return C.astype(np.float32)


# ----------------------------------------------------------------------------
# Host-side edge preparation (indexing only)
# ----------------------------------------------------------------------------
def _prepare(f_in, pos, edge_src, edge_dst):
    import ml_dtypes
    core = edge_dst // NODES_PER_CORE
    blk = (edge_dst % NODES_PER_CORE) // NB
    drel = (edge_dst % NODES_PER_CORE) % NB

    recf_all, recg_all = [], []
    for c in range(NCORES):
        recf = np.zeros((BLOCKS, 128, T_B, 16), dtype=np.float32)
        recg = np.zeros((BLOCKS, 128, T_B, 8), dtype=np.float32)
        recg[..., 3:6] = 1.0          # pad edges: distinct endpoints, r > 0
        recg[..., 6] = PAD_DREL       # pad edges: no destination match
        sel_c = np.nonzero(core == c)[0]
        b_c = blk[sel_c]
        order = np.argsort(b_c, kind='stable')
        sel_c = sel_c[order]
        b_c = b_c[order]
        counts = np.bincount(b_c, minlength=BLOCKS)
        if counts.max() > T_B * NB:
            raise RuntimeError(f"block overflow: {counts.max()} > {T_B * NB}")
        starts = np.concatenate([[0], np.cumsum(counts)])
        pos_in_blk = np.arange(sel_c.size) - starts[b_c]
        lane = pos_in_blk % 128
        tile_i = pos_in_blk // 128
        e = sel_c
        recf[b_c, lane, tile_i, :] = f_in[edge_src[e], :]
        recg[b_c, lane, tile_i, 0:3] = pos[edge_src[e], :]
        recg[b_c, lane, tile_i, 3:6] = pos[edge_dst[e], :]
        recg[b_c, lane, tile_i, 6] = drel[e].astype(np.float32)
        recf_all.append(recf.astype(ml_dtypes.bfloat16))
        recg_all.append(recg)
    return recf_all, recg_all


# ----------------------------------------------------------------------------
# Device kernel builder
# ----------------------------------------------------------------------------
def _build_bass():
    import concourse.bass as bass
    import concourse.bacc as bacc
    import concourse.tile as tile
    from concourse import mybir

    f32 = mybir.dt.float32
    bf16 = mybir.dt.bfloat16
    AF = mybir.ActivationFunctionType
    ALU = mybir.AluOpType

    def view(base_ap, extra_off, free_dims):
        return bass.AP(tensor=base_ap.tensor,
                       offset=base_ap.offset + extra_off,
                       ap=[list(base_ap.ap[0])] + [list(d) for d in free_dims])

    nc = bacc.Bacc()
    recf_d = nc.declare_dram_parameter("recf", [BLOCKS, 128, T_B, 16], bf16, isOutput=False)
    recg_d = nc.declare_dram_parameter("recg", [BLOCKS, 128, T_B, 8], f32, isOutput=False)
    crbd_d = nc.declare_dram_parameter("crbd", [128, RG * 17], f32, isOutput=False)
    cbias_d = nc.declare_dram_parameter("cbias", [128, 1], f32, isOutput=False)
    fold1_d = nc.declare_dram_parameter("fold1", [128, 16], f32, isOutput=False)
    fold2_d = nc.declare_dram_parameter("fold2", [128, 16], f32, isOutput=False)
    ident_d = nc.declare_dram_parameter("ident", [128, 128], f32, isOutput=False)
    nodeidx_d = nc.declare_dram_parameter("nodeidx", [128, 128], f32, isOutput=False)
    out_d = nc.declare_dram_parameter("out", [BLOCKS, 128, 16], f32, isOutput=True)

    inv_h = float(1.0 / RBF_H)

    with ExitStack() as ctx:
        tc = ctx.enter_context(tile.TileContext(nc))
        singles = ctx.enter_context(tc.tile_pool(name="singles", bufs=1))
        loadp = ctx.enter_context(tc.tile_pool(name="loads", bufs=2))
        workp = ctx.enter_context(tc.tile_pool(name="work", bufs=2))
        bigp = ctx.enter_context(tc.tile_pool(name="big", bufs=2))
        psp = ctx.enter_context(tc.tile_pool(name="ps", bufs=2, space="PSUM"))
        psg = ctx.enter_context(tc.tile_pool(name="psg", bufs=2, space="PSUM"))

        ident = singles.tile([128, 128], f32)
        nc.sync.dma_start(out=ident[:], in_=ident_d[:])
        nodeidx = singles.tile([128, 128], f32)
        nc.sync.dma_start(out=nodeidx[:], in_=nodeidx_d[:])
        crbd = singles.tile([128, RG * 17], f32)
        nc.sync.dma_start(out=crbd[:], in_=crbd_d[:])
        cbias = singles.tile([128, 1], f32)
        nc.sync.dma_start(out=cbias[:], in_=cbias_d[:])
        fold1 = singles.tile([128, 16], f32)
        nc.sync.dma_start(out=fold1[:], in_=fold1_d[:])
        fold2 = singles.tile([128, 16], f32)
        nc.sync.dma_start(out=fold2[:], in_=fold2_d[:])

        for b in range(BLOCKS):
            rf = loadp.tile([128, T_B, 16], bf16, tag="rf")
            nc.gpsimd.dma_start(out=rf[:], in_=recf_d[b])
            rg = loadp.tile([128, T_B, 8], f32, tag="rg")
            nc.gpsimd.dma_start(out=rg[:], in_=recg_d[b])

            # ---------------- geometry ----------------
            vec = workp.tile([128, T_B, 3], f32, tag="vec")
            nc.vector.tensor_tensor(vec[:], rg[:, :, 3:6], rg[:, :, 0:3], ALU.subtract)
            vv = workp.tile([128, T_B, 3], f32, tag="vv")
            nc.vector.tensor_tensor(vv[:], vec[:], vec[:], ALU.mult)
            r2 = workp.tile([128, T_B, 1], f32, tag="r2")
            nc.vector.tensor_reduce(r2[:], vv[:], axis=mybir.AxisListType.X, op=ALU.add)
            r = workp.tile([128, T_B, 1], f32, tag="r")
            nc.scalar.activation(r[:], r2[:], AF.Sqrt)
            rinv = workp.tile([128, T_B, 1], f32, tag="rinv")
            nc.vector.reciprocal(rinv[:], r[:])
            u = workp.tile([128, T_B, 3], f32, tag="u")
            nc.vector.tensor_tensor(u[:], vec[:], rinv[:].to_broadcast([128, T_B, 3]), ALU.mult)

            sh = workp.tile([128, T_B, 9], f32, tag="sh")
            nc.gpsimd.memset(sh[:, :, 0:1], 1.0)
            nc.vector.tensor_copy(sh[:, :, 1:3], u[:, :, 1:3])
            nc.vector.tensor_copy(sh[:, :, 3:4], u[:, :, 0:1])
            nc.vector.tensor_tensor(sh[:, :, 4:6], u[:, :, 0:2], u[:, :, 1:3], ALU.mult)
            nc.vector.tensor_tensor(sh[:, :, 7:8], u[:, :, 2:3], u[:, :, 0:1], ALU.mult)
            uu = workp.tile([128, T_B, 3], f32, tag="uu")
            nc.vector.tensor_tensor(uu[:], u[:], u[:], ALU.mult)
            nc.vector.tensor_scalar(sh[:, :, 6:7], uu[:, :, 2:3], 3.0, -1.0, ALU.mult, ALU.add)
            nc.vector.tensor_tensor(sh[:, :, 8:9], uu[:, :, 0:1], uu[:, :, 1:2], ALU.subtract)

            # ---------------- radial weights ----------------
            rrep = workp.tile([128, T_B, M_RBF], f32, tag="rrep")
            nc.vector.tensor_copy(rrep[:], r[:].to_broadcast([128, T_B, M_RBF]))
            w_aos = workp.tile([128, T_B, 17], f32, tag="w_aos")
            for g in range(NGROUPS):
                rT = psp.tile([128, 128], f32, tag="rT")
                nc.tensor.transpose(rT[:], rrep[:, RG * g:RG * (g + 1), :], ident[:])
                sq = workp.tile([128, 128], f32, tag="sq")
                nc.scalar.activation(sq[:], rT[:], AF.Square, bias=cbias[:], scale=inv_h)
                bt = workp.tile([128, 128], f32, tag="bt")
                nc.scalar.activation(bt[:], sq[:], AF.Exp, scale=-1.0)
                wt = psp.tile([RG * 17, 128], f32, tag="wtwb")
                nc.tensor.matmul(wt[:], crbd[:], bt[:], start=True, stop=True)
                wt_s = workp.tile([RG * 17, 128], f32, tag="wt_s")
                nc.scalar.activation(wt_s[:], wt[:], AF.Copy)
                wb = psp.tile([128, RG * 17], f32, tag="wtwb")
                nc.tensor.transpose(wb[:], wt_s[:], ident[0:RG * 17, 0:RG * 17])
                nc.vector.tensor_copy(
                    w_aos[:, RG * g:RG * (g + 1), :],
                    wb[:].rearrange("p (t q) -> p t q", t=RG))

            # ---------------- shw = sh_j * w_p ----------------
            shw = workp.tile([128, T_B, N_SHW], f32, tag="shw")
            sh_b = sh[:]
            w_b = w_aos[:]
            ppos = 0
            while ppos < len(PATHS):
                l1, l2, l3 = PATHS[ppos]
                pcnt = 1
                while (ppos + pcnt < len(PATHS)
                       and PATHS[ppos + pcnt][0] == l1 and PATHS[ppos + pcnt][1] == l2):
                    pcnt += 1
                jw = 2 * l2 + 1
                c0 = SHW_OFF[ppos]
                sh_view = view(sh_b, SL[l2][0], [[9, T_B], [0, pcnt], [1, jw]])
                w_view = view(w_b, ppos, [[17, T_B], [1, pcnt], [0, jw]])
                nc.vector.tensor_tensor(shw[:, :, c0:c0 + pcnt * jw], sh_view, w_view, ALU.mult)
                ppos += pcnt

            # cast shw to bf16 for the product stage
            shw16 = workp.tile([128, T_B, N_SHW], bf16, tag="shw16")
            nc.vector.tensor_copy(shw16[:], shw[:])

            # ---------------- W = f_i * shw ----------------
            W = bigp.tile([128, T_B, W_COLS], bf16, tag="W")
            rf_b = rf[:]
            shw_b = shw16[:]
            wcol = 0
            for l1 in range(4):
                ni = 2 * l1 + 1
                sl1 = S_L1[l1]
                f_view = view(rf_b, SL[l1][0], [[16, T_B], [1, ni], [0, sl1]])
                shw_view = view(shw_b, SHW_L1_START[l1], [[N_SHW, T_B], [0, ni], [1, sl1]])
                nc.vector.tensor_tensor(W[:, :, wcol:wcol + ni * sl1], f_view, shw_view, ALU.mult)
                wcol += ni * sl1

            # ---------------- SEL + segment matmul ----------------
            sel = bigp.tile([128, T_B, 128], bf16, tag="sel")
            ni_view = view(nodeidx[:], 0, [[0, T_B], [1, 128]])
            nc.vector.tensor_tensor(
                sel[:], ni_view, rg[:, :, 6:7].to_broadcast([128, T_B, 128]),
                ALU.is_equal)
            G = psg.tile([128, W_COLS], f32, tag="G")
            for t in range(T_B):
                nc.tensor.matmul(G[:], sel[:, t, :], W[:, t, :],
                                 start=(t == 0), stop=(t == T_B - 1))

            # ---------------- fold ----------------
            g_s = workp.tile([128, W_COLS], f32, tag="g_s")
            nc.vector.tensor_copy(g_s[:], G[:])
            gt1 = psp.tile([128, 128], f32, tag="fold")
            nc.tensor.transpose(gt1[:], g_s[:, 0:128], ident[:])
            gt2 = psp.tile([W_COLS - 128, 128], f32, tag="fold")
            nc.tensor.transpose(gt2[:], g_s[:, 128:W_COLS], ident[:])
            gt1_s = workp.tile([128, 128], f32, tag="gt1_s")
            nc.scalar.activation(gt1_s[:], gt1[:], AF.Copy)
            gt2_s = workp.tile([W_COLS - 128, 128], f32, tag="gt2_s")
            nc.scalar.activation(gt2_s[:], gt2[:], AF.Copy)
            outT = psp.tile([16, 128], f32, tag="fold")
            nc.tensor.matmul(outT[:], fold1[:, :], gt1_s[:], start=True, stop=False)
            nc.tensor.matmul(outT[:], fold2[0:W_COLS - 128, :], gt2_s[:], start=False, stop=True)
            outT_s = workp.tile([16, 128], f32, tag="outT_s")
            nc.scalar.activation(outT_s[:], outT[:], AF.Copy)
            ob = psp.tile([128, 16], f32, tag="fold")
            nc.tensor.transpose(ob[:], outT_s[:], ident[0:16, 0:16])
            ob_s = workp.tile([128, 16], f32, tag="ob_s")
            nc.vector.tensor_copy(ob_s[:], ob[:])
            nc.gpsimd.dma_start(out=out_d[b], in_=ob_s[:])

    nc.compile()
    return nc


_BASS_CACHE = {}


def kernel(f_in, pos, edge_src, edge_dst, W1, W2):
    from concourse.bass_utils import run_bass_kernel_spmd

    f_in = np.asarray(f_in, dtype=np.float32)
    pos = np.asarray(pos, dtype=np.float32)
    edge_src = np.asarray(edge_src, dtype=np.int32)
    edge_dst = np.asarray(edge_dst, dtype=np.int32)
    W1 = np.asarray(W1, dtype=np.float32)
    W2 = np.asarray(W2, dtype=np.float32)
    E = edge_src.shape[0]
    num_neighbors = E / float(N_NODES)

    Crbf = _fit_rbf(W1, W2)
    crbd = np.zeros((128, RG * 17), dtype=np.float32)
    for t in range(RG):
        crbd[M_RBF * t:M_RBF * (t + 1), 17 * t:17 * (t + 1)] = Crbf
    cbias = np.zeros((128, 1), dtype=np.float32)
    for t in range(RG):
        cbias[M_RBF * t:M_RBF * (t + 1), 0] = -RBF_CEN / RBF_H
    F = _build_fold(num_neighbors)
    fold1 = F[0:128, :].copy()
    fold2 = np.zeros((128, 16), dtype=np.float32)
    fold2[0:W_COLS - 128, :] = F[128:W_COLS, :]
    ident = np.eye(128, dtype=np.float32)
    nodeidx = np.broadcast_to(np.arange(128, dtype=np.float32), (128, 128)).copy()

    recf_all, recg_all = _prepare(f_in, pos, edge_src, edge_dst)

    if 'nc' not in _BASS_CACHE:
        _BASS_CACHE['nc'] = _build_bass()
    nc = _BASS_CACHE['nc']

    in_maps = []
    for c in range(NCORES):
        in_maps.append({
            "recf": recf_all[c],
            "recg": recg_all[c],
            "crbd": crbd,
            "cbias": cbias,
            "fold1": fold1,
            "fold2": fold2,
            "ident": ident,
            "nodeidx": nodeidx,
        })
    import os
    trace = bool(os.environ.get("E3_TRACE"))
    res = run_bass_kernel_spmd(nc, in_maps, list(range(NCORES)), trace=trace)
    if trace:
        print("HW exec time:", res.exec_time_ns, "ns")
        if res.profile_json:
            print("profile_json:", res.profile_json)
    out = np.zeros((NPAD, 16), dtype=np.float32)
    for c in range(NCORES):
        out[c * NODES_PER_CORE:(c + 1) * NODES_PER_CORE, :] = \
            np.asarray(res.results[c]["out"]).reshape(NODES_PER_CORE, 16)
    return out[:N_NODES]


# revision 11
# speedup vs baseline: 1.1172x; 1.1172x over previous
"""E(3)-equivariant graph convolution (e3nn-style) on 8 Trainium2 NeuronCores.

Contract: kernel(**inputs) takes the FULL inputs of nn_E3Conv (f_in [N,16],
pos [N,3], edge_src [E], edge_dst [E], W1 [8,64], W2 [64,17]) and returns the
FULL output [N,16] float32.

Sharding (from the sharding hint, hardcoded): nodes are sharded contiguously
across the 8 cores (2560 per core); each edge is assigned to the core owning
its destination node; the host materializes per-edge source records (the
halo-exchange/gather step of a distributed GNN) sorted by destination block;
each core computes per-edge geometry, spherical harmonics, radial weights and
the tensor product on device and scatter-adds into its local nodes.

Device pipeline per 128-node block (36 edge-tiles of 128 edges):
  - geometry + raw spherical harmonics on VectorE,
  - the radial weight function (a function of one scalar r) evaluated as a
    32-center gaussian RBF fit of the reference MLP: PE transpose of r,
    ScalarE Square+Exp for the gaussians, PE matmul against the fitted
    coefficients, PE transpose back,
  - tensor-product expansion columns W[e, 243] = f_i * sh_j * w_p as four
    grouped elementwise products on VectorE (bf16),
  - scatter-add via a selection-matrix matmul (SEL[128e,128n]^T @ W) that
    accumulates all 36 tiles of a block into one PSUM tile G[128n, 243],
  - constant fold G @ F -> out[128, 16] via PE transposes + matmul.
"""

import numpy as np
from contextlib import ExitStack
from math import factorial

# ----------------------------------------------------------------------------
# e3nn constants (replicated from the reference definition; self-contained)
# ----------------------------------------------------------------------------
MAX_RADIUS = 6.0
N_BASIS = 8
N_NODES = 20000
SILU_MOM = 1.6790

SL = [(0, 1), (1, 4), (4, 9), (9, 16)]
PATHS = [(0, 0, 0), (0, 1, 1), (0, 2, 2),
         (1, 0, 1), (1, 1, 0), (1, 1, 2), (1, 2, 1), (1, 2, 3),
         (2, 0, 2), (2, 1, 1), (2, 1, 3), (2, 2, 0), (2, 2, 2),
         (3, 0, 3), (3, 1, 2), (3, 2, 1), (3, 2, 3)]
_K_OUT = {0: 3, 1: 5, 2: 5, 3: 4}
PATH_W = [float(np.sqrt((2 * l3 + 1) / _K_OUT[l3])) for (_, _, l3) in PATHS]


def _cg(j1, m1, j2, m2, j3, m3):
    if m1 + m2 != m3:
        return 0.0
    f = factorial
    pre = ((2 * j3 + 1) * f(j3 + j1 - j2) * f(j3 - j1 + j2) * f(j1 + j2 - j3) / f(j1 + j2 + j3 + 1)) ** 0.5
    pre *= (f(j3 + m3) * f(j3 - m3) * f(j1 - m1) * f(j1 + m1) * f(j2 - m2) * f(j2 + m2)) ** 0.5
    s = 0.0
    for k in range(max(0, j2 - j3 - m1, j1 - j3 + m2), min(j1 + j2 - j3, j1 - m1, j2 + m2) + 1):
        s += (-1.0) ** k / (f(k) * f(j1 + j2 - j3 - k) * f(j1 - m1 - k) * f(j2 + m2 - k) * f(j3 - j2 + m1 + k) * f(j3 - j1 - m2 + k))
    return pre * s


def _q(l):
    q = np.zeros((2 * l + 1, 2 * l + 1), dtype=np.complex128)
    for m in range(-l, 0):
        q[l + m, l + abs(m)] = 2 ** -0.5
        q[l + m, l - abs(m)] = -1j * 2 ** -0.5
    q[l, l] = 1.0
    for m in range(1, l + 1):
        q[l + m, l + abs(m)] = (-1) ** m * 2 ** -0.5
        q[l + m, l - abs(m)] = 1j * (-1) ** m * 2 ** -0.5
    return (-1j) ** l * q


def _w3j(l1, l2, l3):
    C = np.zeros((2 * l1 + 1, 2 * l2 + 1, 2 * l3 + 1))
    for m1 in range(-l1, l1 + 1):
        for m2 in range(-l2, l2 + 1):
            m3 = m1 + m2
            if -l3 <= m3 <= l3:
                C[m1 + l1, m2 + l2, m3 + l3] = _cg(l1, m1, l2, m2, l3, m3)
    Q1, Q2, Q3 = _q(l1), _q(l2), _q(l3)
    Ra = np.einsum('ai,bj,ck,ijk->abc', Q1, Q2, np.conj(Q3), C.astype(np.complex128))
    Rb = np.einsum('ai,bj,ck,ijk->abc', Q1, Q2, Q3, C.astype(np.complex128))
    R = Ra if np.abs(Ra.imag).sum() <= np.abs(Rb.imag).sum() else Rb
    R = R.real
    return (R / np.linalg.norm(R)).astype(np.float64)


W3J = {p: _w3j(*p) for p in set(PATHS)}

# device sh (raw) = [1, uy, uz, ux, ux*uy, uy*uz, 3uz^2-1, uz*ux, ux^2-uy^2]
_s3, _s5, _s15 = np.sqrt(3.0), np.sqrt(5.0), np.sqrt(15.0)
SH_CONST = np.array([1.0, _s3, _s3, _s3, _s15, _s15, _s5 / 2.0, _s15, _s15 / 2.0])

# ----------------------------------------------------------------------------
# Sharding / layout geometry
# ----------------------------------------------------------------------------
NCORES = 8
NB = 128
BLOCKS = 20
NODES_PER_CORE = NB * BLOCKS          # 2560
NPAD = NCORES * NODES_PER_CORE        # 20480
T_B = 36                              # edge tiles per node block (padded)
RG = 4                                # tiles per radial transpose group
NGROUPS = T_B // RG                   # 9
M_RBF = 32
PAD_DREL = 200.0

SHW_COLS = []
SHW_OFF = []
for _pi, (_l1, _l2, _l3) in enumerate(PATHS):
    SHW_OFF.append(len(SHW_COLS))
    for _j in range(2 * _l2 + 1):
        SHW_COLS.append((_pi, _j))
N_SHW = len(SHW_COLS)                 # 57

L1_PATHS = {l1: [pi for pi, p in enumerate(PATHS) if p[0] == l1] for l1 in range(4)}
S_L1 = {l1: sum(2 * PATHS[pi][1] + 1 for pi in L1_PATHS[l1]) for l1 in range(4)}
SHW_L1_START = {l1: SHW_OFF[L1_PATHS[l1][0]] for l1 in range(4)}
W_COLS = sum((2 * l1 + 1) * S_L1[l1] for l1 in range(4))  # 243


def _build_fold(num_neighbors):
    F = np.zeros((W_COLS, 16), dtype=np.float64)
    col = 0
    for l1 in range(4):
        for i in range(2 * l1 + 1):
            for pi in L1_PATHS[l1]:
                p_l1, p_l2, p_l3 = PATHS[pi]
                C = W3J[(p_l1, p_l2, p_l3)]
                for j in range(2 * p_l2 + 1):
                    jg = SL[p_l2][0] + j
                    for k in range(2 * p_l3 + 1):
                        kg = SL[p_l3][0] + k
                        F[col, kg] += C[i, j, k] * SH_CONST[jg]
                    col += 1
    assert col == W_COLS
    return (F / np.sqrt(num_neighbors)).astype(np.float32)


# ----------------------------------------------------------------------------
# Radial RBF fit
# ----------------------------------------------------------------------------
RBF_LO, RBF_HI = -0.3, 6.9
RBF_CEN = np.linspace(RBF_LO, RBF_HI, M_RBF)
RBF_H = (RBF_CEN[1] - RBF_CEN[0]) * 1.45


def _radial_exact(r, W1, W2):
    centers = np.linspace(0.0, MAX_RADIUS, N_BASIS + 2, dtype=np.float32)[1:-1]
    step = np.float32(MAX_RADIUS / (N_BASIS + 1))
    emb = np.exp(-((r[:, None] - centers) / step) ** 2) / 1.12 * np.float32(np.sqrt(N_BASIS))
    z = emb @ W1 / np.float32(np.sqrt(N_BASIS))
    h = SILU_MOM * (z / (1.0 + np.exp(-z)))
    return h @ W2 / np.float32(np.sqrt(W1.shape[1]))


def _fit_rbf(W1, W2):
    g = np.linspace(0.0, 6.6, 6000)
    tgt = _radial_exact(g.astype(np.float32), W1, W2).astype(np.float64)
    tgt = tgt * np.asarray(PATH_W)[None, :]
    B = np.exp(-((g[:, None] - RBF_CEN) / RBF_H) ** 2)
    C, *_ = np.linalg.lstsq(B, tgt, rcond=None)
    return C.astype(np.float32)


# ----------------------------------------------------------------------------
# Host-side edge preparation (indexing only)
# ----------------------------------------------------------------------------
def _prepare(f_in, pos, edge_src, edge_dst):
    import ml_dtypes
    core = edge_dst // NODES_PER_CORE
    blk = (edge_dst % NODES_PER_CORE) // NB
    drel = (edge_dst % NODES_PER_CORE) % NB

    recf_all, recg_all = [], []
    for c in range(NCORES):
        recf = np.zeros((BLOCKS, 128, T_B, 16), dtype=np.float32)
        recg = np.zeros((BLOCKS, 128, T_B, 8), dtype=np.float32)
        recg[..., 3:6] = 1.0          # pad edges: distinct endpoints, r > 0
        recg[..., 6] = PAD_DREL       # pad edges: no destination match
        sel_c = np.nonzero(core == c)[0]
        b_c = blk[sel_c]
        order = np.argsort(b_c, kind='stable')
        sel_c = sel_c[order]
        b_c = b_c[order]
        counts = np.bincount(b_c, minlength=BLOCKS)
        if counts.max() > T_B * NB:
            raise RuntimeError(f"block overflow: {counts.max()} > {T_B * NB}")
        starts = np.concatenate([[0], np.cumsum(counts)])
        pos_in_blk = np.arange(sel_c.size) - starts[b_c]
        lane = pos_in_blk % 128
        tile_i = pos_in_blk // 128
        e = sel_c
        recf[b_c, lane, tile_i, :] = f_in[edge_src[e], :]
        recg[b_c, lane, tile_i, 0:3] = pos[edge_src[e], :]
        recg[b_c, lane, tile_i, 3:6] = pos[edge_dst[e], :]
        recg[b_c, lane, tile_i, 6] = drel[e].astype(np.float32)
        recf_all.append(recf.astype(ml_dtypes.bfloat16))
        recg_all.append(recg)
    return recf_all, recg_all


# ----------------------------------------------------------------------------
# Device kernel builder
# ----------------------------------------------------------------------------
def _build_bass():
    import concourse.bass as bass
    import concourse.bacc as bacc
    import concourse.tile as tile
    from concourse import mybir

    f32 = mybir.dt.float32
    bf16 = mybir.dt.bfloat16
    AF = mybir.ActivationFunctionType
    ALU = mybir.AluOpType

    def view(base_ap, extra_off, free_dims):
        return bass.AP(tensor=base_ap.tensor,
                       offset=base_ap.offset + extra_off,
                       ap=[list(base_ap.ap[0])] + [list(d) for d in free_dims])

    nc = bacc.Bacc()
    recf_d = nc.declare_dram_parameter("recf", [BLOCKS, 128, T_B, 16], bf16, isOutput=False)
    recg_d = nc.declare_dram_parameter("recg", [BLOCKS, 128, T_B, 8], f32, isOutput=False)
    crbd_d = nc.declare_dram_parameter("crbd", [128, RG * 17], f32, isOutput=False)
    cbias_d = nc.declare_dram_parameter("cbias", [128, 1], f32, isOutput=False)
    fold1_d = nc.declare_dram_parameter("fold1", [128, 16], f32, isOutput=False)
    fold2_d = nc.declare_dram_parameter("fold2", [128, 16], f32, isOutput=False)
    ident_d = nc.declare_dram_parameter("ident", [128, 128], f32, isOutput=False)
    nodeidx_d = nc.declare_dram_parameter("nodeidx", [128, 128], f32, isOutput=False)
    out_d = nc.declare_dram_parameter("out", [BLOCKS, 128, 16], f32, isOutput=True)

    inv_h = float(1.0 / RBF_H)

    with ExitStack() as ctx:
        tc = ctx.enter_context(tile.TileContext(nc))
        singles = ctx.enter_context(tc.tile_pool(name="singles", bufs=1))
        loadp = ctx.enter_context(tc.tile_pool(name="loads", bufs=3))
        workp = ctx.enter_context(tc.tile_pool(name="work", bufs=3))
        bigp = ctx.enter_context(tc.tile_pool(name="big", bufs=3))
        psp = ctx.enter_context(tc.tile_pool(name="ps", bufs=1, space="PSUM"))
        psw = ctx.enter_context(tc.tile_pool(name="psw", bufs=4, space="PSUM"))
        psr = ctx.enter_context(tc.tile_pool(name="psr", bufs=1, space="PSUM"))
        psg = ctx.enter_context(tc.tile_pool(name="psg", bufs=2, space="PSUM"))

        ident = singles.tile([128, 128], f32)
        nc.sync.dma_start(out=ident[:], in_=ident_d[:])
        nodeidx = singles.tile([128, 128], f32)
        nc.sync.dma_start(out=nodeidx[:], in_=nodeidx_d[:])
        crbd = singles.tile([128, RG * 17], f32)
        nc.sync.dma_start(out=crbd[:], in_=crbd_d[:])
        cbias = singles.tile([128, 1], f32)
        nc.sync.dma_start(out=cbias[:], in_=cbias_d[:])
        fold1 = singles.tile([128, 16], f32)
        nc.sync.dma_start(out=fold1[:], in_=fold1_d[:])
        fold2 = singles.tile([128, 16], f32)
        nc.sync.dma_start(out=fold2[:], in_=fold2_d[:])

        for b in range(BLOCKS):
            rf = loadp.tile([128, T_B, 16], bf16, tag="rf")
            nc.gpsimd.dma_start(out=rf[:], in_=recf_d[b])
            rg = loadp.tile([128, T_B, 8], f32, tag="rg")
            nc.gpsimd.dma_start(out=rg[:], in_=recg_d[b])

            # ---------------- geometry ----------------
            vec = workp.tile([128, T_B, 3], f32, tag="vec")
            nc.vector.tensor_tensor(vec[:], rg[:, :, 3:6], rg[:, :, 0:3], ALU.subtract)
            vv = workp.tile([128, T_B, 3], f32, tag="vv")
            nc.vector.tensor_tensor(vv[:], vec[:], vec[:], ALU.mult)
            r2 = workp.tile([128, T_B, 1], f32, tag="r2")
            nc.vector.tensor_reduce(r2[:], vv[:], axis=mybir.AxisListType.X, op=ALU.add)
            r = workp.tile([128, T_B, 1], f32, tag="r")
            nc.scalar.activation(r[:], r2[:], AF.Sqrt)
            rinv = workp.tile([128, T_B, 1], f32, tag="rinv")
            nc.vector.reciprocal(rinv[:], r[:])
            u = workp.tile([128, T_B, 3], f32, tag="u")
            nc.vector.tensor_tensor(u[:], vec[:], rinv[:].to_broadcast([128, T_B, 3]), ALU.mult)

            sh = workp.tile([128, T_B, 9], f32, tag="sh")
            nc.gpsimd.memset(sh[:, :, 0:1], 1.0)
            nc.vector.tensor_copy(sh[:, :, 1:3], u[:, :, 1:3])
            nc.vector.tensor_copy(sh[:, :, 3:4], u[:, :, 0:1])
            nc.vector.tensor_tensor(sh[:, :, 4:6], u[:, :, 0:2], u[:, :, 1:3], ALU.mult)
            nc.vector.tensor_tensor(sh[:, :, 7:8], u[:, :, 2:3], u[:, :, 0:1], ALU.mult)
            uu = workp.tile([128, T_B, 3], f32, tag="uu")
            nc.vector.tensor_tensor(uu[:], u[:], u[:], ALU.mult)
            nc.vector.tensor_scalar(sh[:, :, 6:7], uu[:, :, 2:3], 3.0, -1.0, ALU.mult, ALU.add)
            nc.vector.tensor_tensor(sh[:, :, 8:9], uu[:, :, 0:1], uu[:, :, 1:2], ALU.subtract)

            # ---------------- radial weights ----------------
            rrep = workp.tile([128, T_B, M_RBF], f32, tag="rrep")
            nc.gpsimd.tensor_copy(rrep[:], r[:].to_broadcast([128, T_B, M_RBF]))
            w_aos = workp.tile([128, T_B, 17], f32, tag="w_aos")
            for g in range(NGROUPS):
                rT = psr.tile([128, 128], f32, tag="rT")
                nc.tensor.transpose(rT[:], rrep[:, RG * g:RG * (g + 1), :], ident[:])
                sq = workp.tile([128, 128], f32, tag="sq")
                nc.scalar.activation(sq[:], rT[:], AF.Square, bias=cbias[:], scale=inv_h)
                bt = workp.tile([128, 128], f32, tag="bt")
                nc.scalar.activation(bt[:], sq[:], AF.Exp, scale=-1.0)
                wt = psw.tile([RG * 17, 128], f32, tag="wtwb")
                nc.tensor.matmul(wt[:], crbd[:], bt[:], start=True, stop=True)
                wt_s = workp.tile([RG * 17, 128], f32, tag="wt_s")
                nc.scalar.activation(wt_s[:], wt[:], AF.Copy)
                wb = psw.tile([128, RG * 17], f32, tag="wtwb")
                nc.tensor.transpose(wb[:], wt_s[:], ident[0:RG * 17, 0:RG * 17])
                nc.vector.tensor_copy(
                    w_aos[:, RG * g:RG * (g + 1), :],
                    wb[:].rearrange("p (t q) -> p t q", t=RG))

            # ---------------- shw = sh_j * w_p ----------------
            shw = workp.tile([128, T_B, N_SHW], bf16, tag="shw")
            sh_b = sh[:]
            w_b = w_aos[:]
            ppos = 0
            while ppos < len(PATHS):
                l1, l2, l3 = PATHS[ppos]
                pcnt = 1
                while (ppos + pcnt < len(PATHS)
                       and PATHS[ppos + pcnt][0] == l1 and PATHS[ppos + pcnt][1] == l2):
                    pcnt += 1
                jw = 2 * l2 + 1
                c0 = SHW_OFF[ppos]
                sh_view = view(sh_b, SL[l2][0], [[9, T_B], [0, pcnt], [1, jw]])
                w_view = view(w_b, ppos, [[17, T_B], [1, pcnt], [0, jw]])
                nc.vector.tensor_tensor(shw[:, :, c0:c0 + pcnt * jw], sh_view, w_view, ALU.mult)
                ppos += pcnt

            # ---------------- W = f_i * shw ----------------
            W = bigp.tile([128, T_B, W_COLS], bf16, tag="W")
            rf_b = rf[:]
            shw_b = shw[:]
            wcol = 0
            for l1 in range(4):
                ni = 2 * l1 + 1
                sl1 = S_L1[l1]
                f_view = view(rf_b, SL[l1][0], [[16, T_B], [1, ni], [0, sl1]])
                shw_view = view(shw_b, SHW_L1_START[l1], [[N_SHW, T_B], [0, ni], [1, sl1]])
                nc.vector.tensor_tensor(W[:, :, wcol:wcol + ni * sl1], f_view, shw_view, ALU.mult)
                wcol += ni * sl1

            # ---------------- SEL + segment matmul ----------------
            sel = bigp.tile([128, T_B, 128], bf16, tag="sel")
            ni_view = view(nodeidx[:], 0, [[0, T_B], [1, 128]])
            nc.vector.tensor_tensor(
                sel[:], ni_view, rg[:, :, 6:7].to_broadcast([128, T_B, 128]),
                ALU.is_equal)
            G = psg.tile([128, W_COLS], f32, tag="G")
            for t in range(T_B):
                nc.tensor.matmul(G[:], sel[:, t, :], W[:, t, :],
                                 start=(t == 0), stop=(t == T_B - 1))

            # ---------------- fold ----------------
            g_s = workp.tile([128, W_COLS], f32, tag="g_s")
            nc.scalar.activation(g_s[:], G[:], AF.Copy)
            gt1 = psp.tile([128, 128], f32, tag="fold")
            nc.tensor.transpose(gt1[:], g_s[:, 0:128], ident[:])
            gt2 = psp.tile([W_COLS - 128, 128], f32, tag="fold")
            nc.tensor.transpose(gt2[:], g_s[:, 128:W_COLS], ident[:])
            gt1_s = workp.tile([128, 128], f32, tag="gt1_s")
            nc.scalar.activation(gt1_s[:], gt1[:], AF.Copy)
            gt2_s = workp.tile([W_COLS - 128, 128], f32, tag="gt2_s")
            nc.scalar.activation(gt2_s[:], gt2[:], AF.Copy)
            outT = psp.tile([16, 128], f32, tag="fold")
            nc.tensor.matmul(outT[:], fold1[:, :], gt1_s[:], start=True, stop=False)
            nc.tensor.matmul(outT[:], fold2[0:W_COLS - 128, :], gt2_s[:], start=False, stop=True)
            outT_s = workp.tile([16, 128], f32, tag="outT_s")
            nc.scalar.activation(outT_s[:], outT[:], AF.Copy)
            ob = psp.tile([128, 16], f32, tag="fold")
            nc.tensor.transpose(ob[:], outT_s[:], ident[0:16, 0:16])
            ob_s = workp.tile([128, 16], f32, tag="ob_s")
            nc.scalar.activation(ob_s[:], ob[:], AF.Copy)
            nc.gpsimd.dma_start(out=out_d[b], in_=ob_s[:])

    nc.compile()
    return nc


_BASS_CACHE = {}


def kernel(f_in, pos, edge_src, edge_dst, W1, W2):
    from concourse.bass_utils import run_bass_kernel_spmd

    f_in = np.asarray(f_in, dtype=np.float32)
    pos = np.asarray(pos, dtype=np.float32)
    edge_src = np.asarray(edge_src, dtype=np.int32)
    edge_dst = np.asarray(edge_dst, dtype=np.int32)
    W1 = np.asarray(W1, dtype=np.float32)
    W2 = np.asarray(W2, dtype=np.float32)
    E = edge_src.shape[0]
    num_neighbors = E / float(N_NODES)

    Crbf = _fit_rbf(W1, W2)
    crbd = np.zeros((128, RG * 17), dtype=np.float32)
    for t in range(RG):
        crbd[M_RBF * t:M_RBF * (t + 1), 17 * t:17 * (t + 1)] = Crbf
    cbias = np.zeros((128, 1), dtype=np.float32)
    for t in range(RG):
        cbias[M_RBF * t:M_RBF * (t + 1), 0] = -RBF_CEN / RBF_H
    F = _build_fold(num_neighbors)
    fold1 = F[0:128, :].copy()
    fold2 = np.zeros((128, 16), dtype=np.float32)
    fold2[0:W_COLS - 128, :] = F[128:W_COLS, :]
    ident = np.eye(128, dtype=np.float32)
    nodeidx = np.broadcast_to(np.arange(128, dtype=np.float32), (128, 128)).copy()

    recf_all, recg_all = _prepare(f_in, pos, edge_src, edge_dst)

    if 'nc' not in _BASS_CACHE:
        _BASS_CACHE['nc'] = _build_bass()
    nc = _BASS_CACHE['nc']

    in_maps = []
    for c in range(NCORES):
        in_maps.append({
            "recf": recf_all[c],
            "recg": recg_all[c],
            "crbd": crbd,
            "cbias": cbias,
            "fold1": fold1,
            "fold2": fold2,
            "ident": ident,
            "nodeidx": nodeidx,
        })
    import os
    trace = bool(os.environ.get("E3_TRACE"))
    res = run_bass_kernel_spmd(nc, in_maps, list(range(NCORES)), trace=trace)
    if trace:
        print("HW exec time:", res.exec_time_ns, "ns")
        if res.profile_json:
            print("profile_json:", res.profile_json)
    out = np.zeros((NPAD, 16), dtype=np.float32)
    for c in range(NCORES):
        out[c * NODES_PER_CORE:(c + 1) * NODES_PER_CORE, :] = \
            np.asarray(res.results[c]["out"]).reshape(NODES_PER_CORE, 16)
    return out[:N_NODES]
